# revision 15
# baseline (speedup 1.0000x reference)
"""Trainium2 Bass kernel for CrossEgoContrastive (SigLIP-style loss).

Computation (reference):
    x [4096, 1024] -> h = gelu(x@w1+b1) -> p = h@w2+b2 -> z = p/||p||
    logits = z@z.T * exp(log_scale) + logit_bias           [4096, 4096]
    loss   = mean_r( sum_j softplus(-m1*logits) ),  m1 = +1 same batch else -1
    returns (z.reshape(256,16,256), loss, logits)

Strategy: data-parallel shard of the N=4096 rows across 8 cores (512 rows
each). Each core runs the MLP on its row block (weights replicated),
normalizes, AllGathers z (on-chip collective), computes its [512, 4096]
row-block of logits, and accumulates softplus row sums on the ACT engine.

Loss identity used (softplus(-t) - softplus(t) = -t):
    sum_r sum_j softplus(-m1*logits) =
        sum_all softplus(logits) - scale * sum_b ||sum_{r in b} z_r||^2 - 16*N*bias
so no masks are needed; the same-batch correction reduces to batch sums of z.
"""

import numpy as np

import concourse.bacc as bacc
import concourse.mybir as mybir
import concourse.tile as tile
from concourse.masks import make_identity
from concourse.bass_utils import run_bass_kernel_spmd

F32 = mybir.dt.float32
F32R = mybir.dt.float32r

N_CORES = 8
B, A, D, H, P = 256, 16, 1024, 1024, 256
N = B * A            # 4096
NL = N // N_CORES    # 512 rows per core
AF = mybir.ActivationFunctionType


def _patched_act_tables(orig_fn):
    """Restrict Exp/Ln to their shared table set so the table-load chooser
    doesn't alternate between exp_and_others and natural_log every chunk
    (34 loads x 1.3us observed). The emitted set id stays a valid index
    into act_info.json; natural_log_exp_and_others genuinely contains both."""
    def fn(arch):
        tabs = orig_fn(arch)
        out = {}
        for name, funcs in tabs.items():
            f = set(funcs)
            if name != "natural_log_exp_and_others":
                f.discard(AF.Exp)
                f.discard(AF.Ln)
            out[name] = f
        return out
    return fn


def build_nc(sim_mode=False, use_f32r=True):
    """Emit the per-core Bass program. sim_mode substitutes Gelu (unsupported
    by CoreSim) with Tanh so the dataflow can be validated in simulation
    against a numpy mirror. Softplus is computed as Ln(1 + Exp(t)) because
    this compiler's ACT tables have no softplus entry; Exp and Ln share one
    table set so the logits phase needs no table swaps.

    float32r discipline: the BIR verifier requires every f32r matmul input
    to be produced "rounded" — either DMA-loaded as f32r end-to-end or
    written by a compute instruction into an f32r-typed tile. Big-matmul
    operands (w1, w2, xT, hT, zTs, zTf) are f32r; small helper matmuls
    (ones broadcasts/reductions) stay plain fp32."""
    gelu_f = AF.Tanh if sim_mode else AF.Gelu
    MMDT = F32R if use_f32r else F32

    def mmcast(ap):
        return ap.bitcast(F32R) if use_f32r else ap

    orig_tables = bacc.get_activation_tables
    bacc.get_activation_tables = _patched_act_tables(orig_tables)
    try:
        return _build_nc_body(sim_mode, gelu_f, MMDT, mmcast)
    finally:
        bacc.get_activation_tables = orig_tables


def _build_nc_body(sim_mode, gelu_f, MMDT, mmcast):
    nc = bacc.Bacc("TRN2", target_bir_lowering=False, debug=False,
                   num_devices=N_CORES)

    # I/O
    xbt = nc.dram_tensor("xbt", [D, NL], F32, kind="ExternalInput").ap()
    w1 = nc.dram_tensor("w1", [D, H], F32, kind="ExternalInput").ap()
    w2 = nc.dram_tensor("w2", [H, P], F32, kind="ExternalInput").ap()
    b1s = nc.dram_tensor("b1s", [128, H // 128], F32, kind="ExternalInput").ap()
    b2s = nc.dram_tensor("b2s", [128, P // 128], F32, kind="ExternalInput").ap()
    sb = nc.dram_tensor("sb", [1, 2], F32, kind="ExternalInput").ap()  # [scale, bias]
    lg = nc.dram_tensor("lg", [NL, N], F32, kind="ExternalOutput").ap()
    emb = nc.dram_tensor("emb", [NL, P], F32, kind="ExternalOutput").ap()
    t1s = nc.dram_tensor("t1s", [1, 1], F32, kind="ExternalOutput").ap()
    cs = nc.dram_tensor("cs", [1, 1], F32, kind="ExternalOutput").ap()

    KD = D // 128   # 8 embed chunks
    KH = H // 128   # 8 hidden chunks
    KP = P // 128   # 2 proj chunks
    MC = NL // 128  # 4 local row chunks
    JC = N // 512   # 8 global column chunks

    with tile.TileContext(nc) as tc:
        with (
            tc.tile_pool(name="const", bufs=1) as const,
            tc.tile_pool(name="big", bufs=1) as big,
            tc.tile_pool(name="work", bufs=3) as work,
            tc.tile_pool(name="lgp", bufs=4) as lgp,
            tc.tile_pool(name="spp", bufs=2) as spp,
            tc.tile_pool(name="mm", bufs=4, space="PSUM") as mm,
            tc.tile_pool(name="tp", bufs=2, space="PSUM") as tp,
            tc.tile_pool(name="misc", bufs=2, space="PSUM") as misc,
            tc.tile_pool(name="dram", bufs=1, space="DRAM") as dram,
        ):
            # ---- constants / inputs into SBUF ----
            ident = const.tile([128, 128], F32)
            make_identity(nc, ident[:])
            ones_c = const.tile([128, 1], F32)    # column of ones (K=128 reduce)
            nc.vector.memset(ones_c[:], 1.0)
            ones_r = const.tile([1, 128], F32)    # row of ones (K=1 broadcast)
            nc.vector.memset(ones_r[:], 1.0)

            w1_sb = big.tile([128, KD, H], MMDT)
            nc.sync.dma_start(out=w1_sb[:],
                              in_=mmcast(w1.rearrange("(kc p) h -> p kc h", p=128)))
            w2_sb = big.tile([128, KH, P], MMDT)
            nc.sync.dma_start(out=w2_sb[:],
                              in_=mmcast(w2.rearrange("(kc p) h -> p kc h", p=128)))
            b1_sb = const.tile([128, KH], F32)
            nc.sync.dma_start(out=b1_sb[:], in_=b1s)
            b2_sb = const.tile([128, KP], F32)
            nc.sync.dma_start(out=b2_sb[:], in_=b2s)
            sb_sb = const.tile([1, 2], F32)
            nc.sync.dma_start(out=sb_sb[:], in_=sb)
            xT_sb = big.tile([128, KD, NL], MMDT)
            nc.sync.dma_start(out=xT_sb[:],
                              in_=mmcast(xbt.rearrange("(kc p) n -> p kc n", p=128)))

            # broadcast [scale, bias] to all partitions
            ps_sv = misc.tile([128, 2], F32, tag="misc")
            nc.tensor.matmul(ps_sv[:], ones_r[:], sb_sb[:],
                             start=True, stop=True)
            sv_sb = const.tile([128, 2], F32)
            nc.vector.tensor_copy(sv_sb[:], ps_sv[:])
            scale_col = sv_sb[:, 0:1]
            bias_col = sv_sb[:, 1:2]

            # ---- MLP layer 1: hT = gelu(w1.T @ xT + b1) ----
            hT_sb = big.tile([128, KH, NL], MMDT)
            for hc in range(KH):
                ps_h = mm.tile([128, NL], F32, tag="mm")
                for kc in range(KD):
                    nc.tensor.matmul(
                        ps_h[:],
                        w1_sb[:, kc, 128 * hc:128 * (hc + 1)],
                        xT_sb[:, kc, :],
                        start=(kc == 0), stop=(kc == KD - 1),
                    )
                nc.scalar.activation(hT_sb[:, hc, :], ps_h[:], gelu_f,
                                     bias=b1_sb[:, hc:hc + 1], scale=1.0)

            # ---- MLP layer 2: pT = w2.T @ hT + b2 ----
            pT_sb = work.tile([128, KP, NL], F32, tag="pT")
            for pc in range(KP):
                ps_p = mm.tile([128, NL], F32, tag="mm")
                for kc in range(KH):
                    nc.tensor.matmul(
                        ps_p[:],
                        w2_sb[:, kc, 128 * pc:128 * (pc + 1)],
                        hT_sb[:, kc, :],
                        start=(kc == 0), stop=(kc == KH - 1),
                    )
                nc.scalar.activation(pT_sb[:, pc, :], ps_p[:], AF.Identity,
                                     bias=b2_sb[:, pc:pc + 1], scale=1.0)

            # ---- normalize: zT = pT / ||p||, zTs = scale * zT ----
            sq_sb = work.tile([128, KP, NL], F32, tag="sq")
            for pc in range(KP):
                nc.vector.tensor_mul(sq_sb[:, pc, :], pT_sb[:, pc, :], pT_sb[:, pc, :])
            ps_n = misc.tile([1, NL], F32, tag="misc")
            for pc in range(KP):
                nc.tensor.matmul(ps_n[:], ones_c[:], sq_sb[:, pc, :],
                                 start=(pc == 0), stop=(pc == KP - 1))
            norm_sb = work.tile([1, NL], F32, tag="norm")
            nc.scalar.activation(norm_sb[:], ps_n[:], AF.Sqrt)
            rn_sb = work.tile([1, NL], F32, tag="rn")
            nc.vector.reciprocal(rn_sb[:], norm_sb[:])
            ps_bc = misc.tile([128, NL], F32, tag="misc")
            nc.tensor.matmul(ps_bc[:], ones_r[:], rn_sb[:],
                             start=True, stop=True)
            rnb_sb = work.tile([128, NL], F32, tag="rnb")
            nc.vector.tensor_copy(rnb_sb[:], ps_bc[:])

            zT_sb = work.tile([128, KP, NL], F32, tag="zT")
            zTs_sb = work.tile([128, KP, NL], MMDT, tag="zTs")
            for pc in range(KP):
                nc.vector.tensor_mul(zT_sb[:, pc, :], pT_sb[:, pc, :], rnb_sb[:])
                nc.vector.tensor_scalar_mul(zTs_sb[:, pc, :], zT_sb[:, pc, :],
                                            scale_col)

            # ---- AllGather z (unscaled, transposed layout) ----
            zg_in = dram.tile([P, NL], F32)
            for pc in range(KP):
                nc.sync.dma_start(out=zg_in[128 * pc:128 * (pc + 1), :],
                                  in_=zT_sb[:, pc, :])
            zg_out = dram.tile([N_CORES, P, NL], F32, addr_space="Shared")
            nc.gpsimd.collective_compute(
                "AllGather",
                mybir.AluOpType.bypass,
                replica_groups=[list(range(N_CORES))],
                ins=[zg_in.opt()],
                outs=[zg_out.opt()],
            )
            zTf_sb = big.tile([128, KP, N_CORES, NL], MMDT)
            for kc in range(KP):
                nc.sync.dma_start(
                    out=zTf_sb[:, kc],
                    in_=mmcast(zg_out[:, 128 * kc:128 * (kc + 1), :]
                               .rearrange("i p n -> p i n")),
                )

            # ---- embeddings out: transpose zT -> z rows, DMA out ----
            z_sb = work.tile([128, MC, P], F32, tag="z")
            for pc in range(KP):
                for rc in range(MC):
                    ps_t = tp.tile([128, 128], F32, tag="tp")
                    nc.tensor.transpose(ps_t[:], zT_sb[:, pc, 128 * rc:128 * (rc + 1)],
                                        ident[:])
                    nc.vector.tensor_copy(z_sb[:, rc, 128 * pc:128 * (pc + 1)], ps_t[:])
            nc.sync.dma_start(out=emb.rearrange("(rc p) c -> p rc c", p=128),
                              in_=z_sb[:])

            # ---- batch sums correction: cs = sum_b ||sum_{r in b} z_r||^2 ----
            sbt_sb = work.tile([128, KP, B // N_CORES], F32, tag="sbt")
            for pc in range(KP):
                nc.vector.reduce_sum(
                    out=sbt_sb[:, pc, :],
                    in_=zT_sb[:, pc, :].rearrange("p (b t) -> p b t", t=A),
                    axis=mybir.AxisListType.X,
                )
            sbq_sb = work.tile([128, KP, B // N_CORES], F32, tag="sbq")
            for pc in range(KP):
                nc.vector.tensor_mul(sbq_sb[:, pc, :], sbt_sb[:, pc, :], sbt_sb[:, pc, :])
            ps_c = misc.tile([1, B // N_CORES], F32, tag="misc")
            for pc in range(KP):
                nc.tensor.matmul(ps_c[:], ones_c[:], sbq_sb[:, pc, :],
                                 start=(pc == 0), stop=(pc == KP - 1))
            cs_sb = work.tile([1, 1], F32, tag="cs")
            nc.vector.reduce_sum(out=cs_sb[:], in_=ps_c[:], axis=mybir.AxisListType.X)
            nc.sync.dma_start(out=cs, in_=cs_sb[:])

            # ---- logits row-block + softplus row sums ----
            spa_sb = work.tile([128, MC, JC], F32, tag="spa")
            for mc in range(MC):
                for jc in range(JC):
                    ps_l = mm.tile([128, 512], F32, tag="mm")
                    for kc in range(KP):
                        nc.tensor.matmul(
                            ps_l[:],
                            zTs_sb[:, kc, 128 * mc:128 * (mc + 1)],
                            zTf_sb[:, kc, jc, :],
                            start=(kc == 0), stop=(kc == KP - 1),
                        )
                    lg_sb = lgp.tile([128, 512], F32, tag="lg")
                    nc.vector.tensor_scalar_add(lg_sb[:], ps_l[:], bias_col)
                    nc.sync.dma_start(
                        out=lg[128 * mc:128 * (mc + 1), 512 * jc:512 * (jc + 1)],
                        in_=lg_sb[:],
                    )
                    sp_u = spp.tile([128, 512], F32, tag="spu")
                    nc.scalar.activation(sp_u[:], ps_l[:], AF.Exp,
                                         bias=bias_col, scale=1.0)
                    sp_scr = spp.tile([128, 512], F32, tag="sp")
                    nc.scalar.activation(sp_scr[:], sp_u[:], AF.Ln,
                                         bias=1.0, scale=1.0,
                                         accum_out=spa_sb[:, mc, jc:jc + 1])

            # ---- total softplus sum ----
            t1_sb = work.tile([128, MC], F32, tag="t1")
            for mc in range(MC):
                nc.vector.reduce_sum(out=t1_sb[:, mc:mc + 1], in_=spa_sb[:, mc, :],
                                     axis=mybir.AxisListType.X)
            ps_s = misc.tile([1, MC], F32, tag="misc")
            nc.tensor.matmul(ps_s[:], ones_c[:], t1_sb[:],
                             start=True, stop=True)
            t1t_sb = work.tile([1, 1], F32, tag="t1t")
            nc.vector.reduce_sum(out=t1t_sb[:], in_=ps_s[:], axis=mybir.AxisListType.X)
            nc.sync.dma_start(out=t1s, in_=t1t_sb[:])

    nc.compile()
    return nc


_NC_CACHE = {}


def get_nc(sim_mode=False, use_f32r=True):
    key = (sim_mode, use_f32r)
    if key not in _NC_CACHE:
        _NC_CACHE[key] = build_nc(sim_mode=sim_mode, use_f32r=use_f32r)
    return _NC_CACHE[key]


def make_in_maps(inputs):
    x = np.ascontiguousarray(
        np.asarray(inputs["agent_embeddings"], dtype=np.float32).reshape(N, D))
    w1 = np.ascontiguousarray(np.asarray(inputs["w1"], dtype=np.float32))
    w2 = np.ascontiguousarray(np.asarray(inputs["w2"], dtype=np.float32))
    b1 = np.asarray(inputs["b1"], dtype=np.float32)
    b2 = np.asarray(inputs["b2"], dtype=np.float32)
    scale = float(np.exp(np.asarray(inputs["log_scale"], dtype=np.float32)))
    bias = float(np.asarray(inputs["logit_bias"], dtype=np.float32))

    b1s = np.ascontiguousarray(b1.reshape(H // 128, 128).T)
    b2s = np.ascontiguousarray(b2.reshape(P // 128, 128).T)
    sbv = np.array([[scale, bias]], dtype=np.float32)

    in_maps = []
    for c in range(N_CORES):
        xb = x[c * NL:(c + 1) * NL]
        in_maps.append({
            "xbt": np.ascontiguousarray(xb.T),
            "w1": w1,
            "w2": w2,
            "b1s": b1s,
            "b2s": b2s,
            "sb": sbv,
        })
    return in_maps, scale, bias


def assemble(results, scale, bias):
    logits = np.concatenate([r["lg"] for r in results], axis=0)
    embeddings = np.concatenate([r["emb"] for r in results], axis=0)
    embeddings = embeddings.reshape(B, A, P)
    t1 = sum(float(r["t1s"][0, 0]) for r in results)
    csum = sum(float(r["cs"][0, 0]) for r in results)
    loss_sum = t1 - scale * csum - A * N * bias
    loss = np.float32(loss_sum / N)
    return embeddings, loss, logits


def kernel(**inputs):
    nc = get_nc(sim_mode=False, use_f32r=True)
    in_maps, scale, bias = make_in_maps(inputs)
    res = run_bass_kernel_spmd(nc, in_maps, core_ids=list(range(N_CORES)))
    return assemble(res.results, scale, bias)


# revision 17
# speedup vs baseline: 58.6662x; 58.6662x over previous
"""Trainium2 Bass kernel for CrossEgoContrastive (SigLIP-style loss).

Computation (reference):
    x [4096, 1024] -> h = gelu(x@w1+b1) -> p = h@w2+b2 -> z = p/||p||
    logits = z@z.T * exp(log_scale) + logit_bias           [4096, 4096]
    loss   = mean_r( sum_j softplus(-m1*logits) ),  m1 = +1 same batch else -1
    returns (z.reshape(256,16,256), loss, logits)

Strategy: data-parallel shard of the N=4096 rows across 8 cores (512 rows
each). Each core runs the MLP on its row block (weights replicated),
normalizes, AllGathers z (on-chip collective), computes its [512, 4096]
row-block of logits, and accumulates softplus row sums on the ACT engine.

Loss identity used (softplus(-t) - softplus(t) = -t):
    sum_r sum_j softplus(-m1*logits) =
        sum_all softplus(logits) - scale * sum_b ||sum_{r in b} z_r||^2 - 16*N*bias
so no masks are needed; the same-batch correction reduces to batch sums of z.
"""

import numpy as np

import concourse.bacc as bacc
import concourse.mybir as mybir
import concourse.tile as tile
from concourse.masks import make_identity
from concourse.bass_utils import run_bass_kernel_spmd

F32 = mybir.dt.float32
F32R = mybir.dt.float32r

N_CORES = 8
B, A, D, H, P = 256, 16, 1024, 1024, 256
N = B * A            # 4096
NL = N // N_CORES    # 512 rows per core
AF = mybir.ActivationFunctionType


def _patched_act_tables(orig_fn):
    """Restrict Exp/Ln to their shared table set so the table-load chooser
    doesn't alternate between exp_and_others and natural_log every chunk
    (34 loads x 1.3us observed). The emitted set id stays a valid index
    into act_info.json; natural_log_exp_and_others genuinely contains both."""
    def fn(arch):
        tabs = orig_fn(arch)
        out = {}
        for name, funcs in tabs.items():
            f = set(funcs)
            if name != "natural_log_exp_and_others":
                f.discard(AF.Exp)
                f.discard(AF.Ln)
            out[name] = f
        return out
    return fn


def build_nc(sim_mode=False, use_f32r=True):
    """Emit the per-core Bass program. sim_mode substitutes Gelu (unsupported
    by CoreSim) with Tanh so the dataflow can be validated in simulation
    against a numpy mirror. Softplus is computed as Ln(1 + Exp(t)) because
    this compiler's ACT tables have no softplus entry; Exp and Ln share one
    table set so the logits phase needs no table swaps.

    float32r discipline: the BIR verifier requires every f32r matmul input
    to be produced "rounded" — either DMA-loaded as f32r end-to-end or
    written by a compute instruction into an f32r-typed tile. Big-matmul
    operands (w1, w2, xT, hT, zTs, zTf) are f32r; small helper matmuls
    (ones broadcasts/reductions) stay plain fp32."""
    gelu_f = AF.Tanh if sim_mode else AF.Gelu
    MMDT = F32R if use_f32r else F32

    def mmcast(ap):
        return ap.bitcast(F32R) if use_f32r else ap

    orig_tables = bacc.get_activation_tables
    bacc.get_activation_tables = _patched_act_tables(orig_tables)
    try:
        return _build_nc_body(sim_mode, gelu_f, MMDT, mmcast)
    finally:
        bacc.get_activation_tables = orig_tables


def _build_nc_body(sim_mode, gelu_f, MMDT, mmcast):
    nc = bacc.Bacc("TRN2", target_bir_lowering=False, debug=False,
                   num_devices=N_CORES)

    # I/O
    xbt = nc.dram_tensor("xbt", [D, NL], F32, kind="ExternalInput").ap()
    w1 = nc.dram_tensor("w1", [D, H], F32, kind="ExternalInput").ap()
    w2 = nc.dram_tensor("w2", [H, P], F32, kind="ExternalInput").ap()
    b1s = nc.dram_tensor("b1s", [128, H // 128], F32, kind="ExternalInput").ap()
    b2s = nc.dram_tensor("b2s", [128, P // 128], F32, kind="ExternalInput").ap()
    sb = nc.dram_tensor("sb", [1, 2], F32, kind="ExternalInput").ap()  # [scale, bias]
    lg = nc.dram_tensor("lg", [NL, N], F32, kind="ExternalOutput").ap()
    emb = nc.dram_tensor("emb", [NL, P], F32, kind="ExternalOutput").ap()
    t1s = nc.dram_tensor("t1s", [1, 1], F32, kind="ExternalOutput").ap()
    cs = nc.dram_tensor("cs", [1, 1], F32, kind="ExternalOutput").ap()

    KD = D // 128   # 8 embed chunks
    KH = H // 128   # 8 hidden chunks
    KP = P // 128   # 2 proj chunks
    MC = NL // 128  # 4 local row chunks
    JC = N // 512   # 8 global column chunks

    with tile.TileContext(nc) as tc:
        with (
            tc.tile_pool(name="const", bufs=1) as const,
            tc.tile_pool(name="big", bufs=1) as big,
            tc.tile_pool(name="work", bufs=3) as work,
            tc.tile_pool(name="lgp", bufs=4) as lgp,
            tc.tile_pool(name="spp", bufs=2) as spp,
            tc.tile_pool(name="mm", bufs=4, space="PSUM") as mm,
            tc.tile_pool(name="tp", bufs=2, space="PSUM") as tp,
            tc.tile_pool(name="misc", bufs=2, space="PSUM") as misc,
            tc.tile_pool(name="dram", bufs=1, space="DRAM") as dram,
        ):
            # ---- constants / inputs into SBUF ----
            ident = const.tile([128, 128], F32)
            make_identity(nc, ident[:])
            ones_c = const.tile([128, 1], F32)    # column of ones (K=128 reduce)
            nc.vector.memset(ones_c[:], 1.0)
            ones_r = const.tile([1, 128], F32)    # row of ones (K=1 broadcast)
            nc.vector.memset(ones_r[:], 1.0)

            w1_sb = big.tile([128, KD, H], MMDT)
            nc.sync.dma_start(out=w1_sb[:],
                              in_=mmcast(w1.rearrange("(kc p) h -> p kc h", p=128)))
            w2_sb = big.tile([128, KH, P], MMDT)
            nc.sync.dma_start(out=w2_sb[:],
                              in_=mmcast(w2.rearrange("(kc p) h -> p kc h", p=128)))
            b1_sb = const.tile([128, KH], F32)
            nc.sync.dma_start(out=b1_sb[:], in_=b1s)
            b2_sb = const.tile([128, KP], F32)
            nc.sync.dma_start(out=b2_sb[:], in_=b2s)
            sb_sb = const.tile([1, 2], F32)
            nc.sync.dma_start(out=sb_sb[:], in_=sb)
            xT_sb = big.tile([128, KD, NL], MMDT)
            nc.sync.dma_start(out=xT_sb[:],
                              in_=mmcast(xbt.rearrange("(kc p) n -> p kc n", p=128)))

            # broadcast [scale, bias] to all partitions
            ps_sv = misc.tile([128, 2], F32, tag="misc")
            nc.tensor.matmul(ps_sv[:], ones_r[:], sb_sb[:],
                             start=True, stop=True)
            sv_sb = const.tile([128, 2], F32)
            nc.vector.tensor_copy(sv_sb[:], ps_sv[:])
            scale_col = sv_sb[:, 0:1]
            bias_col = sv_sb[:, 1:2]

            # ---- MLP layer 1: hT = gelu(w1.T @ xT + b1) ----
            hT_sb = big.tile([128, KH, NL], MMDT)
            for hc in range(KH):
                ps_h = mm.tile([128, NL], F32, tag="mm")
                for kc in range(KD):
                    nc.tensor.matmul(
                        ps_h[:],
                        w1_sb[:, kc, 128 * hc:128 * (hc + 1)],
                        xT_sb[:, kc, :],
                        start=(kc == 0), stop=(kc == KD - 1),
                    )
                nc.scalar.activation(hT_sb[:, hc, :], ps_h[:], gelu_f,
                                     bias=b1_sb[:, hc:hc + 1], scale=1.0)

            # ---- MLP layer 2: pT = w2.T @ hT + b2 ----
            pT_sb = work.tile([128, KP, NL], F32, tag="pT")
            for pc in range(KP):
                ps_p = mm.tile([128, NL], F32, tag="mm")
                for kc in range(KH):
                    nc.tensor.matmul(
                        ps_p[:],
                        w2_sb[:, kc, 128 * pc:128 * (pc + 1)],
                        hT_sb[:, kc, :],
                        start=(kc == 0), stop=(kc == KH - 1),
                    )
                nc.scalar.activation(pT_sb[:, pc, :], ps_p[:], AF.Identity,
                                     bias=b2_sb[:, pc:pc + 1], scale=1.0)

            # ---- normalize: zT = pT / ||p||, zTs = scale * zT ----
            sq_sb = work.tile([128, KP, NL], F32, tag="sq")
            for pc in range(KP):
                nc.vector.tensor_mul(sq_sb[:, pc, :], pT_sb[:, pc, :], pT_sb[:, pc, :])
            ps_n = misc.tile([1, NL], F32, tag="misc")
            for pc in range(KP):
                nc.tensor.matmul(ps_n[:], ones_c[:], sq_sb[:, pc, :],
                                 start=(pc == 0), stop=(pc == KP - 1))
            norm_sb = work.tile([1, NL], F32, tag="norm")
            nc.scalar.activation(norm_sb[:], ps_n[:], AF.Sqrt)
            rn_sb = work.tile([1, NL], F32, tag="rn")
            nc.vector.reciprocal(rn_sb[:], norm_sb[:])
            ps_bc = misc.tile([128, NL], F32, tag="misc")
            nc.tensor.matmul(ps_bc[:], ones_r[:], rn_sb[:],
                             start=True, stop=True)
            rnb_sb = work.tile([128, NL], F32, tag="rnb")
            nc.vector.tensor_copy(rnb_sb[:], ps_bc[:])

            zT_sb = work.tile([128, KP, NL], F32, tag="zT")
            zTs_sb = work.tile([128, KP, NL], MMDT, tag="zTs")
            for pc in range(KP):
                nc.vector.tensor_mul(zT_sb[:, pc, :], pT_sb[:, pc, :], rnb_sb[:])
                nc.vector.tensor_scalar_mul(zTs_sb[:, pc, :], zT_sb[:, pc, :],
                                            scale_col)

            # ---- AllGather z (unscaled, transposed layout) ----
            zg_in = dram.tile([P, NL], F32)
            for pc in range(KP):
                nc.sync.dma_start(out=zg_in[128 * pc:128 * (pc + 1), :],
                                  in_=zT_sb[:, pc, :])
            zg_out = dram.tile([N_CORES, P, NL], F32, addr_space="Shared")
            nc.gpsimd.collective_compute(
                "AllGather",
                mybir.AluOpType.bypass,
                replica_groups=[list(range(N_CORES))],
                ins=[zg_in.opt()],
                outs=[zg_out.opt()],
            )
            zTf_sb = big.tile([128, KP, N_CORES, NL], MMDT)
            for kc in range(KP):
                nc.sync.dma_start(
                    out=zTf_sb[:, kc],
                    in_=mmcast(zg_out[:, 128 * kc:128 * (kc + 1), :]
                               .rearrange("i p n -> p i n")),
                )

            # ---- embeddings out: transpose zT -> z rows, DMA out ----
            z_sb = work.tile([128, MC, P], F32, tag="z")
            for pc in range(KP):
                for rc in range(MC):
                    ps_t = tp.tile([128, 128], F32, tag="tp")
                    nc.tensor.transpose(ps_t[:], zT_sb[:, pc, 128 * rc:128 * (rc + 1)],
                                        ident[:])
                    nc.vector.tensor_copy(z_sb[:, rc, 128 * pc:128 * (pc + 1)], ps_t[:])
            nc.sync.dma_start(out=emb.rearrange("(rc p) c -> p rc c", p=128),
                              in_=z_sb[:])

            # ---- batch sums correction: cs = sum_b ||sum_{r in b} z_r||^2 ----
            sbt_sb = work.tile([128, KP, B // N_CORES], F32, tag="sbt")
            for pc in range(KP):
                nc.vector.reduce_sum(
                    out=sbt_sb[:, pc, :],
                    in_=zT_sb[:, pc, :].rearrange("p (b t) -> p b t", t=A),
                    axis=mybir.AxisListType.X,
                )
            sbq_sb = work.tile([128, KP, B // N_CORES], F32, tag="sbq")
            for pc in range(KP):
                nc.vector.tensor_mul(sbq_sb[:, pc, :], sbt_sb[:, pc, :], sbt_sb[:, pc, :])
            ps_c = misc.tile([1, B // N_CORES], F32, tag="misc")
            for pc in range(KP):
                nc.tensor.matmul(ps_c[:], ones_c[:], sbq_sb[:, pc, :],
                                 start=(pc == 0), stop=(pc == KP - 1))
            cs_sb = work.tile([1, 1], F32, tag="cs")
            nc.vector.reduce_sum(out=cs_sb[:], in_=ps_c[:], axis=mybir.AxisListType.X)
            nc.sync.dma_start(out=cs, in_=cs_sb[:])

            # ---- logits row-block + softplus row sums ----
            spa_sb = work.tile([128, MC, JC], F32, tag="spa")
            for mc in range(MC):
                for jc in range(JC):
                    ps_l = mm.tile([128, 512], F32, tag="mm")
                    for kc in range(KP):
                        nc.tensor.matmul(
                            ps_l[:],
                            zTs_sb[:, kc, 128 * mc:128 * (mc + 1)],
                            zTf_sb[:, kc, jc, :],
                            start=(kc == 0), stop=(kc == KP - 1),
                        )
                    lg_sb = lgp.tile([128, 512], F32, tag="lg")
                    nc.vector.tensor_scalar_add(lg_sb[:], ps_l[:], bias_col)
                    nc.sync.dma_start(
                        out=lg[128 * mc:128 * (mc + 1), 512 * jc:512 * (jc + 1)],
                        in_=lg_sb[:],
                    )
                    sp_u = spp.tile([128, 512], F32, tag="spu")
                    nc.scalar.activation(sp_u[:], ps_l[:], AF.Exp,
                                         bias=bias_col, scale=1.0)
                    sp_scr = spp.tile([128, 512], F32, tag="sp")
                    nc.scalar.activation(sp_scr[:], sp_u[:], AF.Ln,
                                         bias=1.0, scale=1.0,
                                         accum_out=spa_sb[:, mc, jc:jc + 1])

            # ---- total softplus sum ----
            t1_sb = work.tile([128, MC], F32, tag="t1")
            for mc in range(MC):
                nc.vector.reduce_sum(out=t1_sb[:, mc:mc + 1], in_=spa_sb[:, mc, :],
                                     axis=mybir.AxisListType.X)
            ps_s = misc.tile([1, MC], F32, tag="misc")
            nc.tensor.matmul(ps_s[:], ones_c[:], t1_sb[:],
                             start=True, stop=True)
            t1t_sb = work.tile([1, 1], F32, tag="t1t")
            nc.vector.reduce_sum(out=t1t_sb[:], in_=ps_s[:], axis=mybir.AxisListType.X)
            nc.sync.dma_start(out=t1s, in_=t1t_sb[:])

    nc.compile()
    return nc


_NC_CACHE = {}


def get_nc(sim_mode=False, use_f32r=True):
    key = (sim_mode, use_f32r)
    if key not in _NC_CACHE:
        _NC_CACHE[key] = build_nc(sim_mode=sim_mode, use_f32r=use_f32r)
    return _NC_CACHE[key]


def make_in_maps(inputs):
    x = np.ascontiguousarray(
        np.asarray(inputs["agent_embeddings"], dtype=np.float32).reshape(N, D))
    w1 = np.ascontiguousarray(np.asarray(inputs["w1"], dtype=np.float32))
    w2 = np.ascontiguousarray(np.asarray(inputs["w2"], dtype=np.float32))
    b1 = np.asarray(inputs["b1"], dtype=np.float32)
    b2 = np.asarray(inputs["b2"], dtype=np.float32)
    scale = float(np.exp(np.asarray(inputs["log_scale"], dtype=np.float32)))
    bias = float(np.asarray(inputs["logit_bias"], dtype=np.float32))

    b1s = np.ascontiguousarray(b1.reshape(H // 128, 128).T)
    b2s = np.ascontiguousarray(b2.reshape(P // 128, 128).T)
    sbv = np.array([[scale, bias]], dtype=np.float32)

    in_maps = []
    for c in range(N_CORES):
        xb = x[c * NL:(c + 1) * NL]
        in_maps.append({
            "xbt": np.ascontiguousarray(xb.T),
            "w1": w1,
            "w2": w2,
            "b1s": b1s,
            "b2s": b2s,
            "sb": sbv,
        })
    return in_maps, scale, bias


def assemble(results, scale, bias):
    logits = np.concatenate([r["lg"] for r in results], axis=0)
    embeddings = np.concatenate([r["emb"] for r in results], axis=0)
    embeddings = embeddings.reshape(B, A, P)
    t1 = sum(float(r["t1s"][0, 0]) for r in results)
    csum = sum(float(r["cs"][0, 0]) for r in results)
    loss_sum = t1 - scale * csum - A * N * bias
    loss = np.float32(loss_sum / N)
    return embeddings, loss, logits


class Runner:
    """Persistent jitted SPMD runner (mirrors bass2jax.run_bass_via_pjrt but
    keeps the compiled callable, device-resident weights, and on-device zero
    outputs so repeat calls skip re-tracing and host zero-transfers)."""

    def __init__(self, nc, n_iters=1):
        import jax
        from jax.sharding import Mesh, PartitionSpec, NamedSharding
        from jax.experimental.shard_map import shard_map
        from concourse import bass2jax

        bass2jax.install_neuronx_cc_hook()
        self.jax = jax
        self.nc = nc
        pname = nc.partition_id_tensor.name if nc.partition_id_tensor else None
        in_names, out_names, out_avals, zero_shapes = [], [], [], []
        for alloc in nc.m.functions[0].allocations:
            if not isinstance(alloc, mybir.MemoryLocationSet):
                continue
            name = alloc.memorylocations[0].name
            if alloc.kind == "ExternalInput":
                if name != pname:
                    in_names.append(name)
            elif alloc.kind == "ExternalOutput":
                shape = tuple(alloc.tensor_shape)
                dtype = mybir.dt.np(alloc.dtype)
                out_names.append(name)
                out_avals.append(jax.core.ShapedArray(shape, dtype))
                zero_shapes.append((shape, dtype))
        self.in_names, self.out_names = in_names, out_names
        self.out_avals, self.zero_shapes = out_avals, zero_shapes
        n_params = len(in_names)
        all_in = list(in_names) + list(out_names)
        if pname is not None:
            all_in.append(pname)
        donate = tuple(range(n_params, n_params + len(out_names)))

        def _body(*args):
            operands = list(args)
            outs = None
            for _ in range(n_iters):
                ops = list(operands)
                if pname is not None:
                    ops.append(bass2jax.partition_id_tensor())
                outs = bass2jax._bass_exec_p.bind(
                    *ops,
                    out_avals=tuple(out_avals),
                    in_names=tuple(all_in),
                    out_names=tuple(out_names),
                    lowering_input_output_aliases=(),
                    sim_require_finite=True,
                    sim_require_nnan=True,
                    nc=nc,
                )
            return tuple(outs)

        devices = jax.devices()[:N_CORES]
        self.mesh = Mesh(np.asarray(devices), ("core",))
        self.sharding = NamedSharding(self.mesh, PartitionSpec("core"))
        self.fn = jax.jit(
            shard_map(_body, mesh=self.mesh,
                      in_specs=(PartitionSpec("core"),) * (n_params + len(out_names)),
                      out_specs=(PartitionSpec("core"),) * len(out_names),
                      check_rep=False),
            donate_argnums=donate, keep_unused=True,
        )

    def make_zeros(self):
        import jax.numpy as jnp
        return [jnp.zeros((N_CORES * s[0], *s[1:]), d, device=self.sharding)
                for (s, d) in self.zero_shapes]

    def put_inputs(self, in_maps):
        return [
            self.jax.device_put(
                np.concatenate([np.asarray(in_maps[c][nm]) for c in range(N_CORES)],
                               axis=0),
                self.sharding)
            for nm in self.in_names
        ]

    def run_dev(self, dev_in):
        outs = self.fn(*dev_in, *self.make_zeros())
        self.jax.block_until_ready(outs)
        return outs

    def run(self, in_maps):
        outs = self.run_dev(self.put_inputs(in_maps))
        return [
            {nm: np.asarray(outs[i]).reshape(N_CORES, *self.out_avals[i].shape)[c]
             for i, nm in enumerate(self.out_names)}
            for c in range(N_CORES)
        ]


_RUNNER = None


def get_runner():
    global _RUNNER
    if _RUNNER is None:
        _RUNNER = Runner(get_nc(sim_mode=False, use_f32r=True))
    return _RUNNER


def kernel(**inputs):
    in_maps, scale, bias = make_in_maps(inputs)
    try:
        results = get_runner().run(in_maps)
    except Exception:
        # fallback: the stock SPMD entry point (same underlying path)
        nc = get_nc(sim_mode=False, use_f32r=True)
        results = run_bass_kernel_spmd(
            nc, in_maps, core_ids=list(range(N_CORES))).results
    return assemble(results, scale, bias)
